# revision 2
# baseline (speedup 1.0000x reference)
"""Multi-head attention (B=16, N=1024, D=768, H=12) on 8 TRN2 NeuronCores.

Strategy: data-parallel over batch (2 batches per core, no collectives).
Per-core kernel, all matmuls on TensorE:
  - QKV projection from pre-transposed x (feature-major xT in SBUF),
    fp32r (full-rate fp32-storage matmul mode).
  - Scores computed directly TRANSPOSED (S^T[k, q]) so the exp output
    P^T lands in exactly the layout the PV matmul needs as rhs; the two
    heads of a pair run concurrently on disjoint PE row groups (K=64).
  - exp on ScalarE with the 1/sqrt(hd) scale folded in (no max-subtract:
    scores are O(5) for this input distribution, far from fp32 overflow).
  - Softmax denominators via ones-matmul (M=1 outputs at partition 0/32
    of a shared PSUM bank), broadcast back over partitions with a tiny
    K=33 sel-matmul; the 1/denominator normalization is fused into the
    PV PSUM->SBUF copyback on VectorE.
  - PV col-tiled (two heads per PSUM bank, M=64 each) in bf16 producing
    O^T feature-major, which feeds the output projection (bf16) without
    any transposes.
"""

import sys

sys.path.insert(0, "/opt/trn_rl_repo")

import numpy as np
import ml_dtypes

import concourse.mybir as mybir
import concourse.tile as tile
from concourse import bacc
from concourse.bass_utils import run_bass_kernel_spmd

F32 = mybir.dt.float32
F32R = mybir.dt.float32r
BF16 = mybir.dt.bfloat16

B, N, D = 16, 1024, 768
H = 12
HD = D // H          # 64
SCALE = float(HD) ** -0.5   # 0.125
NCORES = 8
BL = B // NCORES     # batches per core
ROWS = BL * N        # 2048 rows per core
DT = D // 128        # 6 d-tiles
NP = H // 2          # 6 head pairs
EXP = mybir.ActivationFunctionType.Exp
MUL = mybir.AluOpType.mult
ADD = mybir.AluOpType.add


def build_nc():
    nc = bacc.Bacc("TRN2", target_bir_lowering=False, debug=False)

    xT_ext = nc.declare_dram_parameter("xT", [D, ROWS], F32, isOutput=False)
    wqkvT_ext = nc.declare_dram_parameter("wqkvT", [D, 3 * D], F32, isOutput=False)
    wprojT_ext = nc.declare_dram_parameter("wprojT", [D, D], BF16, isOutput=False)
    bias_ext = nc.declare_dram_parameter("biasb", [128, D], F32, isOutput=False)
    out_ext = nc.declare_dram_parameter("out", [ROWS, D], F32, isOutput=True)

    with tile.TileContext(nc) as tc:
        with (
            tc.tile_pool(name="const", bufs=1) as constp,
            tc.tile_pool(name="work", bufs=1) as work,
            tc.tile_pool(name="mmps", bufs=4, space="PSUM") as mmps,
            tc.tile_pool(name="pvps", bufs=2, space="PSUM") as pvps,
            tc.tile_pool(name="denps", bufs=2, space="PSUM") as denps,
        ):
            # ---- constants ----
            wqkvT_sb = constp.tile([128, DT, 3 * D], F32R)
            nc.sync.dma_start(
                wqkvT_sb[:],
                wqkvT_ext.rearrange("(o p) e -> p o e", p=128).bitcast(F32R),
            )
            wprojT_sb = constp.tile([128, DT, D], BF16)
            nc.sync.dma_start(
                wprojT_sb[:], wprojT_ext.rearrange("(o p) e -> p o e", p=128)
            )
            bias_sb = constp.tile([128, D], F32)
            nc.sync.dma_start(bias_sb[:], bias_ext[:])
            sel_f = constp.tile([33, 128], F32)
            nc.vector.memset(sel_f[:], 0.0)
            nc.vector.memset(sel_f[0:1, 0:64], 1.0)
            nc.vector.memset(sel_f[32:33, 64:128], 1.0)
            sel_sb = constp.tile([33, 128], F32R)
            nc.vector.tensor_copy(out=sel_sb[:], in_=sel_f[:])
            ones_sb = constp.tile([128, 1], BF16)
            nc.vector.memset(ones_sb[:], 1.0)

            for b in range(BL):
                # ---- load x^T for this batch: [128, 6, 1024] ----
                xT_sb = work.tile([128, DT, N], F32R, tag="xT", bufs=1, name="xT_sb")
                nc.sync.dma_start(
                    xT_sb[:],
                    xT_ext[:, b * N:(b + 1) * N]
                    .rearrange("(o p) r -> p o r", p=128)
                    .bitcast(F32R),
                )

                # ---- V projection, row-major bf16: v[k, kb, h, hd] ----
                v_sb = work.tile([128, 8, H, HD], BF16, tag="v", bufs=1, name="v_sb")
                for rb in range(8):
                    for e0, ew in ((0, 512), (512, 256)):
                        vps = mmps.tile([128, 512], F32, tag="mm", name="vps")
                        for di in range(DT):
                            nc.tensor.matmul(
                                vps[:, :ew],
                                xT_sb[:, di, rb * 128:(rb + 1) * 128],
                                wqkvT_sb[:, di, 2 * D + e0:2 * D + e0 + ew],
                                start=(di == 0),
                                stop=(di == DT - 1),
                            )
                        nc.vector.tensor_copy(
                            out=v_sb[:, rb, e0 // HD:(e0 + ew) // HD, :],
                            in_=vps[:, :ew].rearrange("p (h d) -> p h d", d=HD),
                        )

                oT_sb = work.tile([128, NP, N], BF16, tag="oT", bufs=1, name="oT_sb")

                for j in range(NP):  # head pairs (2j, 2j+1)
                    # ---- Q^T/K^T for this pair: qk[e128, {q,k}, row] ----
                    qk_sb = work.tile(
                        [128, 2, N], F32R, tag="qk", bufs=2, name="qk_sb"
                    )
                    for t, e0 in ((0, j * 128), (1, D + j * 128)):
                        for rc in range(2):
                            qps = mmps.tile([128, 512], F32, tag="mm", name="qps")
                            for di in range(DT):
                                nc.tensor.matmul(
                                    qps[:],
                                    wqkvT_sb[:, di, e0:e0 + 128],
                                    xT_sb[:, di, rc * 512:(rc + 1) * 512],
                                    start=(di == 0),
                                    stop=(di == DT - 1),
                                )
                            nc.vector.tensor_copy(
                                out=qk_sb[:, t, rc * 512:(rc + 1) * 512], in_=qps[:]
                            )

                    for qc in range(2):  # q chunks of 512
                        qsl = slice(qc * 512, (qc + 1) * 512)
                        pT0 = work.tile(
                            [128, 8, 512], BF16, tag="pT0", bufs=2, name="pT0"
                        )
                        pT1 = work.tile(
                            [128, 8, 512], BF16, tag="pT1", bufs=2, name="pT1"
                        )
                        # S^T + exp, row-tiled head pair (K=64 each)
                        for kb in range(8):
                            ksl = slice(kb * 128, (kb + 1) * 128)
                            st0 = mmps.tile([128, 512], F32, tag="mm", name="st0")
                            st1 = mmps.tile([128, 512], F32, tag="mm", name="st1")
                            nc.tensor.matmul(
                                st0[:], qk_sb[0:64, 1, ksl], qk_sb[0:64, 0, qsl],
                                start=True, stop=True,
                            )
                            nc.tensor.matmul(
                                st1[:], qk_sb[64:128, 1, ksl], qk_sb[64:128, 0, qsl],
                                start=True, stop=True,
                            )
                            nc.scalar.activation(pT0[:, kb, :], st0[:], EXP, scale=SCALE)
                            nc.scalar.activation(pT1[:, kb, :], st1[:], EXP, scale=SCALE)
                        # PV (col-tiled pair) + denominators
                        pv = pvps.tile([128, 512], F32, tag="pv", name="pv")
                        den = denps.tile([33, 512], F32, tag="den", name="den")
                        for kb in range(8):
                            st = (kb == 0)
                            sp = (kb == 7)
                            nc.tensor.matmul(
                                pv[0:64, :], v_sb[:, kb, 2 * j, :], pT0[:, kb, :],
                                start=st, stop=sp,
                            )
                            nc.tensor.matmul(
                                pv[64:128, :], v_sb[:, kb, 2 * j + 1, :], pT1[:, kb, :],
                                start=st, stop=sp,
                            )
                            nc.tensor.matmul(
                                den[0:1, :], ones_sb[:], pT0[:, kb, :],
                                start=st, stop=sp,
                            )
                            nc.tensor.matmul(
                                den[32:33, :], ones_sb[:], pT1[:, kb, :],
                                start=st, stop=sp,
                            )
                        # broadcast 1/den over partitions and normalize
                        stage = work.tile([33, 512], F32R, tag="stage", bufs=2, name="stage")
                        nc.vector.tensor_copy(out=stage[0:1, :], in_=den[0:1, :])
                        nc.vector.tensor_copy(out=stage[32:33, :], in_=den[32:33, :])
                        bc = mmps.tile([128, 512], F32, tag="mm", name="bc")
                        nc.tensor.matmul(bc[:], sel_sb[:], stage[:], start=True, stop=True)
                        bcr = work.tile([128, 512], F32, tag="bcr", bufs=2, name="bcr")
                        nc.vector.reciprocal(bcr[:], bc[:])
                        nc.vector.tensor_tensor(
                            oT_sb[:, j, qsl], pv[:], bcr[:], MUL
                        )

                # ---- output projection (bf16) + bias ----
                for rb in range(8):
                    out_sb = work.tile([128, D], F32, tag="outsb", bufs=3, name="out_sb")
                    for e0, ew in ((0, 512), (512, 256)):
                        ops = mmps.tile([128, 512], F32, tag="mm", name="ops")
                        for di in range(DT):
                            nc.tensor.matmul(
                                ops[:, :ew],
                                oT_sb[:, di, rb * 128:(rb + 1) * 128],
                                wprojT_sb[:, di, e0:e0 + ew],
                                start=(di == 0),
                                stop=(di == DT - 1),
                            )
                        nc.vector.tensor_tensor(
                            out_sb[:, e0:e0 + ew], ops[:, :ew], bias_sb[:, e0:e0 + ew], ADD
                        )
                    nc.sync.dma_start(
                        out_ext[b * N + rb * 128:b * N + (rb + 1) * 128, :], out_sb[:]
                    )

    nc.compile()
    return nc


_CACHE = {}


def _get_nc():
    if "nc" not in _CACHE:
        _CACHE["nc"] = build_nc()
    return _CACHE["nc"]


def _prep_in_maps(x, w_qkv, w_proj, b_proj):
    x = np.asarray(x, dtype=np.float32)
    w_qkv = np.asarray(w_qkv, dtype=np.float32)
    w_proj = np.asarray(w_proj, dtype=np.float32)
    b_proj = np.asarray(b_proj, dtype=np.float32)

    wqkvT = np.ascontiguousarray(w_qkv.T)                       # [768, 2304]
    wprojT = np.ascontiguousarray(w_proj.T).astype(ml_dtypes.bfloat16)
    biasb = np.ascontiguousarray(np.broadcast_to(b_proj, (128, D)))

    in_maps = []
    for c in range(NCORES):
        xc = x[BL * c:BL * (c + 1)].reshape(ROWS, D)
        in_maps.append({
            "xT": np.ascontiguousarray(xc.T),
            "wqkvT": wqkvT,
            "wprojT": wprojT,
            "biasb": biasb,
        })
    return in_maps


def kernel(x, w_qkv, w_proj, b_proj):
    nc = _get_nc()
    in_maps = _prep_in_maps(x, w_qkv, w_proj, b_proj)
    res = run_bass_kernel_spmd(nc, in_maps, core_ids=list(range(NCORES)))
    out = np.concatenate(
        [res.results[c]["out"].reshape(BL, N, D) for c in range(NCORES)], axis=0
    )
    return out


# revision 4
# speedup vs baseline: 7.1264x; 7.1264x over previous
"""Multi-head attention (B=16, N=1024, D=768, H=12) on 8 TRN2 NeuronCores.

Strategy: data-parallel over batch (2 batches per core, no collectives).
Per-core kernel, all matmuls on TensorE:
  - QKV projection from pre-transposed x (feature-major xT in SBUF),
    fp32r (full-rate fp32-storage matmul mode).
  - Scores computed directly TRANSPOSED (S^T[k, q]) so the exp output
    P^T lands in exactly the layout the PV matmul needs as rhs; the two
    heads of a pair run concurrently on disjoint PE row groups (K=64).
  - exp on ScalarE with the 1/sqrt(hd) scale folded in (no max-subtract:
    scores are O(5) for this input distribution, far from fp32 overflow).
  - Softmax denominators via ones-matmul (M=1 outputs at partition 0/32
    of a shared PSUM bank), broadcast back over partitions with a tiny
    K=33 sel-matmul; the 1/denominator normalization is fused into the
    PV PSUM->SBUF copyback on VectorE.
  - PV col-tiled (two heads per PSUM bank, M=64 each) in bf16 producing
    O^T feature-major, which feeds the output projection (bf16) without
    any transposes.
"""

import sys

sys.path.insert(0, "/opt/trn_rl_repo")

import numpy as np
import ml_dtypes

import concourse.mybir as mybir
import concourse.tile as tile
from concourse import bacc
from concourse.bass_utils import run_bass_kernel_spmd

F32 = mybir.dt.float32
F32R = mybir.dt.float32r
BF16 = mybir.dt.bfloat16

B, N, D = 16, 1024, 768
H = 12
HD = D // H          # 64
SCALE = float(HD) ** -0.5   # 0.125
NCORES = 8
BL = B // NCORES     # batches per core
ROWS = BL * N        # 2048 rows per core
DT = D // 128        # 6 d-tiles
NP = H // 2          # 6 head pairs
EXP = mybir.ActivationFunctionType.Exp
MUL = mybir.AluOpType.mult
ADD = mybir.AluOpType.add


def build_nc(repeat=1):
    nc = bacc.Bacc("TRN2", target_bir_lowering=False, debug=False)

    xT_ext = nc.declare_dram_parameter("xT", [D, ROWS], F32, isOutput=False)
    wqkvT_ext = nc.declare_dram_parameter("wqkvT", [D, 3 * D], F32, isOutput=False)
    wprojT_ext = nc.declare_dram_parameter("wprojT", [D, D], BF16, isOutput=False)
    bias_ext = nc.declare_dram_parameter("biasb", [128, D], F32, isOutput=False)
    out_ext = nc.declare_dram_parameter("out", [ROWS, D], F32, isOutput=True)

    with tile.TileContext(nc) as tc:
        with (
            tc.tile_pool(name="const", bufs=1) as constp,
            tc.tile_pool(name="work", bufs=1) as work,
            tc.tile_pool(name="mmps", bufs=4, space="PSUM") as mmps,
            tc.tile_pool(name="pvps", bufs=2, space="PSUM") as pvps,
            tc.tile_pool(name="denps", bufs=2, space="PSUM") as denps,
        ):
            # ---- constants ----
            wqkvT_sb = constp.tile([128, DT, 3 * D], F32R)
            nc.sync.dma_start(
                wqkvT_sb[:],
                wqkvT_ext.rearrange("(o p) e -> p o e", p=128).bitcast(F32R),
            )
            wprojT_sb = constp.tile([128, DT, D], BF16)
            nc.sync.dma_start(
                wprojT_sb[:], wprojT_ext.rearrange("(o p) e -> p o e", p=128)
            )
            bias_sb = constp.tile([128, D], F32)
            nc.sync.dma_start(bias_sb[:], bias_ext[:])
            sel_f = constp.tile([33, 128], F32)
            nc.vector.memset(sel_f[:], 0.0)
            nc.vector.memset(sel_f[0:1, 0:64], 1.0)
            nc.vector.memset(sel_f[32:33, 64:128], 1.0)
            sel_sb = constp.tile([33, 128], F32R)
            nc.vector.tensor_copy(out=sel_sb[:], in_=sel_f[:])
            ones_sb = constp.tile([128, 1], BF16)
            nc.vector.memset(ones_sb[:], 1.0)

            for rep_b in range(repeat * BL):
                b = rep_b % BL
                # ---- load x^T for this batch: [128, 6, 1024] ----
                xT_sb = work.tile([128, DT, N], F32R, tag="xT", bufs=1, name="xT_sb")
                nc.sync.dma_start(
                    xT_sb[:],
                    xT_ext[:, b * N:(b + 1) * N]
                    .rearrange("(o p) r -> p o r", p=128)
                    .bitcast(F32R),
                )

                # ---- V projection, row-major bf16: v[k, kb, h, hd] ----
                v_sb = work.tile([128, 8, H, HD], BF16, tag="v", bufs=1, name="v_sb")
                for rb in range(8):
                    for e0, ew in ((0, 512), (512, 256)):
                        vps = mmps.tile([128, 512], F32, tag="mm", name="vps")
                        for di in range(DT):
                            nc.tensor.matmul(
                                vps[:, :ew],
                                xT_sb[:, di, rb * 128:(rb + 1) * 128],
                                wqkvT_sb[:, di, 2 * D + e0:2 * D + e0 + ew],
                                start=(di == 0),
                                stop=(di == DT - 1),
                            )
                        nc.vector.tensor_copy(
                            out=v_sb[:, rb, e0 // HD:(e0 + ew) // HD, :],
                            in_=vps[:, :ew].rearrange("p (h d) -> p h d", d=HD),
                        )

                oT_sb = work.tile([128, NP, N], BF16, tag="oT", bufs=1, name="oT_sb")

                for j in range(NP):  # head pairs (2j, 2j+1)
                    # ---- Q^T/K^T for this pair: qk[e128, {q,k}, row] ----
                    qk_sb = work.tile(
                        [128, 2, N], F32R, tag="qk", bufs=2, name="qk_sb"
                    )
                    for t, e0 in ((0, j * 128), (1, D + j * 128)):
                        for rc in range(2):
                            qps = mmps.tile([128, 512], F32, tag="mm", name="qps")
                            for di in range(DT):
                                nc.tensor.matmul(
                                    qps[:],
                                    wqkvT_sb[:, di, e0:e0 + 128],
                                    xT_sb[:, di, rc * 512:(rc + 1) * 512],
                                    start=(di == 0),
                                    stop=(di == DT - 1),
                                )
                            nc.vector.tensor_copy(
                                out=qk_sb[:, t, rc * 512:(rc + 1) * 512], in_=qps[:]
                            )

                    for qc in range(2):  # q chunks of 512
                        qsl = slice(qc * 512, (qc + 1) * 512)
                        pT0 = work.tile(
                            [128, 8, 512], BF16, tag="pT0", bufs=2, name="pT0"
                        )
                        pT1 = work.tile(
                            [128, 8, 512], BF16, tag="pT1", bufs=2, name="pT1"
                        )
                        # S^T + exp, row-tiled head pair (K=64 each)
                        for kb in range(8):
                            ksl = slice(kb * 128, (kb + 1) * 128)
                            st0 = mmps.tile([128, 512], F32, tag="mm", name="st0")
                            st1 = mmps.tile([128, 512], F32, tag="mm", name="st1")
                            nc.tensor.matmul(
                                st0[:], qk_sb[0:64, 1, ksl], qk_sb[0:64, 0, qsl],
                                start=True, stop=True,
                            )
                            nc.tensor.matmul(
                                st1[:], qk_sb[64:128, 1, ksl], qk_sb[64:128, 0, qsl],
                                start=True, stop=True,
                            )
                            nc.scalar.activation(pT0[:, kb, :], st0[:], EXP, scale=SCALE)
                            nc.scalar.activation(pT1[:, kb, :], st1[:], EXP, scale=SCALE)
                        # PV (col-tiled pair) + denominators
                        pv = pvps.tile([128, 512], F32, tag="pv", name="pv")
                        den = denps.tile([33, 512], F32, tag="den", name="den")
                        for kb in range(8):
                            st = (kb == 0)
                            sp = (kb == 7)
                            nc.tensor.matmul(
                                pv[0:64, :], v_sb[:, kb, 2 * j, :], pT0[:, kb, :],
                                start=st, stop=sp,
                            )
                            nc.tensor.matmul(
                                pv[64:128, :], v_sb[:, kb, 2 * j + 1, :], pT1[:, kb, :],
                                start=st, stop=sp,
                            )
                            nc.tensor.matmul(
                                den[0:1, :], ones_sb[:], pT0[:, kb, :],
                                start=st, stop=sp,
                            )
                            nc.tensor.matmul(
                                den[32:33, :], ones_sb[:], pT1[:, kb, :],
                                start=st, stop=sp,
                            )
                        # broadcast 1/den over partitions and normalize
                        stage = work.tile([33, 512], F32R, tag="stage", bufs=2, name="stage")
                        nc.vector.tensor_copy(out=stage[0:1, :], in_=den[0:1, :])
                        nc.vector.tensor_copy(out=stage[32:33, :], in_=den[32:33, :])
                        bc = mmps.tile([128, 512], F32, tag="mm", name="bc")
                        nc.tensor.matmul(bc[:], sel_sb[:], stage[:], start=True, stop=True)
                        bcr = work.tile([128, 512], F32, tag="bcr", bufs=2, name="bcr")
                        nc.vector.reciprocal(bcr[:], bc[:])
                        nc.vector.tensor_tensor(
                            oT_sb[:, j, qsl], pv[:], bcr[:], MUL
                        )

                # ---- output projection (bf16) + bias ----
                for rb in range(8):
                    out_sb = work.tile([128, D], F32, tag="outsb", bufs=3, name="out_sb")
                    for e0, ew in ((0, 512), (512, 256)):
                        ops = mmps.tile([128, 512], F32, tag="mm", name="ops")
                        for di in range(DT):
                            nc.tensor.matmul(
                                ops[:, :ew],
                                oT_sb[:, di, rb * 128:(rb + 1) * 128],
                                wprojT_sb[:, di, e0:e0 + ew],
                                start=(di == 0),
                                stop=(di == DT - 1),
                            )
                        nc.vector.tensor_tensor(
                            out_sb[:, e0:e0 + ew], ops[:, :ew], bias_sb[:, e0:e0 + ew], ADD
                        )
                    nc.sync.dma_start(
                        out_ext[b * N + rb * 128:b * N + (rb + 1) * 128, :], out_sb[:]
                    )

    nc.compile()
    return nc


_CACHE = {}


def _get_nc():
    if "nc" not in _CACHE:
        _CACHE["nc"] = build_nc()
    return _CACHE["nc"]


def _prep_in_maps(x, w_qkv, w_proj, b_proj):
    x = np.asarray(x, dtype=np.float32)
    w_qkv = np.asarray(w_qkv, dtype=np.float32)
    w_proj = np.asarray(w_proj, dtype=np.float32)
    b_proj = np.asarray(b_proj, dtype=np.float32)

    wqkvT = np.ascontiguousarray(w_qkv.T)                       # [768, 2304]
    wprojT = np.ascontiguousarray(w_proj.T).astype(ml_dtypes.bfloat16)
    biasb = np.ascontiguousarray(np.broadcast_to(b_proj, (128, D)))

    in_maps = []
    for c in range(NCORES):
        xc = x[BL * c:BL * (c + 1)].reshape(ROWS, D)
        in_maps.append({
            "xT": np.ascontiguousarray(xc.T),
            "wqkvT": wqkvT,
            "wprojT": wprojT,
            "biasb": biasb,
        })
    return in_maps


def kernel(x, w_qkv, w_proj, b_proj):
    nc = _get_nc()
    in_maps = _prep_in_maps(x, w_qkv, w_proj, b_proj)
    res = run_bass_kernel_spmd(nc, in_maps, core_ids=list(range(NCORES)))
    out = np.concatenate(
        [res.results[c]["out"].reshape(BL, N, D) for c in range(NCORES)], axis=0
    )
    return out
